# revision 7
# baseline (speedup 1.0000x reference)
"""CASCADES adapter (moe_routing) Trainium2 kernel.

Reference math:
    centroid = 0.7*x[:,-1,:] + 0.3*mean_s(x)           [B, IN]
    w        = softmax(cos(centroid, core_keys)/TEMP)  [B, K]
    Lam[b]   = sum_k w[b,k] * core_pool[k]             [B, R, R]
    out      = gate * x @ V^T @ Lam^T @ U^T            [B, S, OUT]
gate is a scalar depending only on U, V, gate_w, gate_b (host-computed).

Restructuring:
    out[b] = xV[b] @ UL[b]^T,   xV = x @ V^T (rank R=8),
    UL[b]  = gate * U @ Lam[b]  [OUT, R]  (tiny, host-computed)
Routing needs only per-batch column sums of x (device-computed in stage 1),
x[:,-1,:] and tiny tensors (host).

Sharding: 8 cores, core c owns batch c//2, S rows [(c%2)*2048, (c%2+1)*2048).

Precision budget: harness gate is rel_err < 2e-2. x is read as bf16 (16
MB/core instead of fp32's 32 MB) and out is written as bf16 (16 MB/core),
each adding ~1e-3 relative noise; V keeps a bf16 hi/lo 2-pass split (free
under the DMA roofline), xv/UL are single-pass bf16. Total rel err ~2e-3.
Memory roofline: 32 MB/core at ~358 GB/s ~= 94 us.
"""

import os
from contextlib import ExitStack

import ml_dtypes
import numpy as np

import concourse.tile as tile
from concourse import bacc, mybir
from concourse.bass_utils import run_bass_kernel_spmd

FP = mybir.dt.float32
BF = mybir.dt.bfloat16
BF_NP = ml_dtypes.bfloat16

B, S, IN, OUT, R, K = 4, 4096, 4096, 4096, 8, 4
NCORES = 8
SSH = S // 2          # 2048: per-core S shard
NI_CH = IN // 128     # 32 contraction chunks
EPS = 1e-8
TEMP = 0.05

# Populated on every kernel() call when KERNEL_TRACE=1.
LAST_STATS: dict = {}

_prog_cache: dict = {}


def _split_hi_lo(a):
    """fp32 array -> (hi, lo) bf16 arrays with hi+lo ~= a (~16-bit mantissa)."""
    a = np.asarray(a, dtype=np.float32)
    hi = a.astype(BF_NP)
    lo = (a - hi.astype(np.float32)).astype(BF_NP)
    return hi, lo


# Stage-1 DMA slabs: chunks per slab (sum = NI_CH = 32). Mostly 4-chunk
# slabs (16 KB partition lines -> few large DMA packets); the last two are
# small so the end-of-kernel tail (last slab's colsum+matmul) is short.
S1_SLABS = [4, 4, 4, 4, 4, 4, 4, 2, 1, 1]


def build_stage1():
    """Per core:
      xv[r, s] = sum_i V[r,i] * x[i, s]     (x bf16, V bf16 hi/lo 2-pass,
                                             fp32 PSUM accumulate)
      cs partials: per-chunk free-axis sums of x (fp32 accum_out on
      ScalarE + VectorE; host adds the partials -> column sums of x)
    Input xbig [128, NI_CH*SSH] bf16: partition-cyclic chunk-major pack of
    the transposed shard: xbig[p, ic*SSH + s] = x_shard[s, ic*128 + p].
    Long contiguous partition lines keep the DMA engines packet-rate
    efficient (4 KB lines cap them at ~294 GB/s aggregate).
    Input vhl [128, 2*NI_CH*R] bf16: [Vh chunks (NI_CH*R) | Vl chunks].
    """
    nc = bacc.Bacc("TRN2", target_bir_lowering=False, debug=False, num_devices=NCORES)
    xbig = nc.dram_tensor("xbig", [128, NI_CH * SSH], BF, kind="ExternalInput").ap()
    vhl = nc.dram_tensor("vhl", [128, 2 * NI_CH * R], BF, kind="ExternalInput").ap()
    xv = nc.dram_tensor("xv", [R, 4 * 512], FP, kind="ExternalOutput").ap()
    cs = nc.dram_tensor("cs", [128, 2 * NI_CH], FP, kind="ExternalOutput").ap()

    with tile.TileContext(nc) as tc:
        with ExitStack() as ctx:
            xin = ctx.enter_context(tc.tile_pool(name="xin", bufs=4))
            scr = ctx.enter_context(tc.tile_pool(name="scr", bufs=2))
            scr2 = ctx.enter_context(tc.tile_pool(name="scr2", bufs=2))
            small = ctx.enter_context(tc.tile_pool(name="small", bufs=1))
            psum = ctx.enter_context(tc.tile_pool(name="psum", bufs=1, space="PSUM"))

            v_sb = small.tile([128, 2 * NI_CH * R], BF)
            nc.sync.dma_start(v_sb[:], vhl[:])
            acc = small.tile([128, 2 * NI_CH], FP)  # 2 partial sums per chunk
            # s-slice sb accumulates at partitions 32*sb..+8, bank sb
            # (PE column tiling: 4 concurrent 128x32 tiles; one accumulation
            # group per PSUM bank - groups are bank-granular).
            xvp = psum.tile([128, 4 * 512], FP)

            NSB = SSH // 512  # 4 rhs slices per pass
            c0 = 0
            for nch in S1_SLABS:
                xt = xin.tile([128, 4 * SSH], BF)
                nc.sync.dma_start(
                    xt[:, 0:nch * SSH],
                    xbig[:, c0 * SSH:(c0 + nch) * SSH])
                for jc in range(nch):
                    ic = c0 + jc
                    xc = xt[:, jc * SSH:(jc + 1) * SSH]
                    # column sums split ScalarE/VectorE by clock ratio (both
                    # are 1x-mode accumulate ops); host adds the partials.
                    sc_t = scr.tile([128, 832], BF)
                    nc.scalar.activation(
                        sc_t[:], xc[:, 0:832], mybir.ActivationFunctionType.Copy,
                        accum_out=acc[:, ic:ic + 1])
                    sc_t2 = scr2.tile([128, 1216], BF)
                    nc.vector.tensor_scalar(
                        sc_t2[:], xc[:, 832:2048], 1.0, None, mybir.AluOpType.mult,
                        mybir.AluOpType.add,
                        accum_out=acc[:, NI_CH + ic:NI_CH + ic + 1])
                    # 2 passes: x@Vh + x@Vl; sb rotates PE column group
                    vh = v_sb[:, ic * R:(ic + 1) * R]
                    vl = v_sb[:, NI_CH * R + ic * R: NI_CH * R + (ic + 1) * R]
                    passes = [vh, vl]
                    for pi, lhsT in enumerate(passes):
                        for sb in range(NSB):
                            nc.tensor.matmul(
                                xvp[32 * sb:32 * sb + R, sb * 512:(sb + 1) * 512],
                                lhsT,
                                xc[:, sb * 512:(sb + 1) * 512],
                                start=(ic == 0 and pi == 0),
                                stop=(ic == NI_CH - 1 and pi == len(passes) - 1),
                                tile_position=(0, 32 * sb),
                            )
                c0 += nch

            xv_sb = small.tile([R, 4 * 512], FP)
            for sb in range(NSB):
                nc.vector.tensor_copy(
                    xv_sb[:, sb * 512:(sb + 1) * 512],
                    xvp[32 * sb:32 * sb + R, sb * 512:(sb + 1) * 512])
            nc.sync.dma_start(xv[:], xv_sb[:])
            nc.sync.dma_start(cs[:], acc[:])

    nc.compile()
    return nc


# Stage-2 output slabs: s-chunks per DMA (sum = 16). 2-chunk slabs give
# 16 KB partition lines; the final singles shorten the last-DMA tail.
S2_SLABS = [2, 2, 2, 2, 2, 2, 2, 1, 1]


def build_stage2():
    """Per core: out[s, o] = sum_r xv[r, s] * ulT[r, o]  (single-pass bf16,
    bf16 output).

    Inputs (xvq [R, SSH] bf16, ulq [R, OUT] bf16) are replicated on-chip
    into all four 32-partition quadrants so matmuls can rotate PE row
    groups (tile_position) - 4 concurrent 32x128 tiles hide the
    per-matmul LDWEIGHTS that otherwise serializes (K=8).

    Output odev [128, 16*OUT] bf16 is partition-cyclic s-chunk-major:
    odev[p, sc*OUT + o] = out[sc*128 + p, o]; the host un-permutes. This
    keeps DMA partition lines at 8-16 KB (packet-rate efficiency).
    """
    nc = bacc.Bacc("TRN2", target_bir_lowering=False, debug=False, num_devices=NCORES)
    xvq = nc.dram_tensor("xvq", [R, SSH], BF, kind="ExternalInput").ap()
    ulq = nc.dram_tensor("ulq", [R, OUT], BF, kind="ExternalInput").ap()
    odev = nc.dram_tensor(
        "odev", [128, (SSH // 128) * OUT], BF, kind="ExternalOutput").ap()

    with tile.TileContext(nc) as tc:
        with ExitStack() as ctx:
            small = ctx.enter_context(tc.tile_pool(name="small", bufs=1))
            ostage = ctx.enter_context(tc.tile_pool(name="ostage", bufs=3))
            psum = ctx.enter_context(tc.tile_pool(name="psum", bufs=2, space="PSUM"))

            xv_sb = small.tile([128, SSH], BF)
            nc.sync.dma_start(xv_sb[0:R, :], xvq[:])
            ul_sb = small.tile([128, OUT], BF)
            nc.sync.dma_start(ul_sb[0:R, :], ulq[:])
            # replicate to quadrants on the idle SWDGE ring
            for q in range(1, 4):
                nc.gpsimd.dma_start(xv_sb[32 * q:32 * q + R, :], xv_sb[0:R, :])
                nc.gpsimd.dma_start(ul_sb[32 * q:32 * q + R, :], ul_sb[0:R, :])

            sc0 = 0
            for nsc in S2_SLABS:
                ot = ostage.tile([128, 2 * OUT], BF)
                for jsc in range(nsc):
                    sc = sc0 + jsc
                    for oh in range(OUT // 2048):  # 2 halves per s-chunk
                        op = psum.tile([128, 2048], FP)  # 4 banks
                        # ob rotates the PE row group every matmul so
                        # LDWEIGHTS+streams of adjacent matmuls overlap. The
                        # very first tile sticks to quadrant 0 so it can start
                        # before the quadrant replication DMAs land (slower
                        # matmuls, but they hide the replication latency).
                        first_tile = (sc == 0 and oh == 0)
                        for ob in range(4):
                            p0 = 0 if first_tile else 32 * ob
                            xh = xv_sb[p0:p0 + R, sc * 128:(sc + 1) * 128]
                            o0 = oh * 2048 + ob * 512
                            uh = ul_sb[p0:p0 + R, o0:o0 + 512]
                            nc.tensor.matmul(
                                op[:, ob * 512:(ob + 1) * 512], xh, uh,
                                start=True, stop=True,
                                tile_position=(p0, 0))
                        od0 = jsc * OUT + oh * 2048
                        # split the PSUM evacuation across both engines
                        nc.vector.tensor_copy(
                            ot[:, od0:od0 + 1024], op[:, 0:1024])
                        nc.scalar.copy(
                            ot[:, od0 + 1024:od0 + 2048], op[:, 1024:2048])
                nc.sync.dma_start(
                    odev[:, sc0 * OUT:(sc0 + nsc) * OUT], ot[:, 0:nsc * OUT])
                sc0 += nsc

    nc.compile()
    return nc


def _get_prog(name, builder):
    if name not in _prog_cache:
        _prog_cache[name] = builder()
    return _prog_cache[name]


def _routing_host(colsum, x_last, V_shared, U_shared, core_pool, core_keys,
                  gate_w, gate_b):
    """All tiny routing math in float64. colsum: [B, IN] sums over S.
    Returns UL[b] = gate * U @ Lam[b]  [B, OUT, R]."""
    m = colsum / S
    xl = x_last.astype(np.float64)
    centroid = 0.7 * xl + 0.3 * m
    cn = centroid / np.maximum(
        np.linalg.norm(centroid, axis=-1, keepdims=True), EPS)
    kn = core_keys.astype(np.float64)
    kn = kn / np.maximum(np.linalg.norm(kn, axis=-1, keepdims=True), EPS)
    sim = cn @ kn.T
    z = sim / TEMP
    z = z - z.max(axis=-1, keepdims=True)
    w = np.exp(z)
    w = w / w.sum(axis=-1, keepdims=True)
    Lam = np.einsum("bk,kij->bij", w, core_pool.astype(np.float64))
    gate_in = np.concatenate([
        U_shared.astype(np.float64).mean(axis=0),
        V_shared.astype(np.float64).mean(axis=1)])
    gate = 1.0 / (1.0 + np.exp(
        -(gate_w.astype(np.float64) @ gate_in + gate_b.astype(np.float64))))
    UL = gate[0] * np.einsum("oj,bjr->bor", U_shared.astype(np.float64), Lam)
    return UL


def kernel(x, V_shared, U_shared, core_pool, core_keys, gate_w, gate_b):
    trace = os.environ.get("KERNEL_TRACE", "") == "1"
    core_ids = list(range(NCORES))

    x = np.asarray(x, dtype=np.float32)
    V_shared = np.asarray(V_shared, dtype=np.float32)
    U_shared = np.asarray(U_shared, dtype=np.float32)
    core_pool = np.asarray(core_pool, dtype=np.float32)
    core_keys = np.asarray(core_keys, dtype=np.float32)
    gate_w = np.asarray(gate_w, dtype=np.float32)
    gate_b = np.asarray(gate_b, dtype=np.float32)

    # ---- host prep: per-core transposed shards, bf16-rounded, packed
    # partition-cyclic chunk-major: xbig[p, ic*SSH + s] = xT[ic*128 + p, s]
    xbs = []
    for c in range(NCORES):
        xs = np.ascontiguousarray(x[c // 2, (c % 2) * SSH:(c % 2 + 1) * SSH, :].T)
        xt = xs.astype(BF_NP)  # [IN, SSH] bf16
        xbs.append(np.ascontiguousarray(
            xt.reshape(NI_CH, 128, SSH).transpose(1, 0, 2).reshape(
                128, NI_CH * SSH)))

    def chunk_major(vmat):  # [R, IN] -> [128, NI_CH*R]
        return np.ascontiguousarray(
            vmat.T.reshape(NI_CH, 128, R).transpose(1, 0, 2).reshape(128, NI_CH * R))

    vh, vl = _split_hi_lo(V_shared)
    vhl = np.concatenate(
        [chunk_major(vh.astype(np.float32)).astype(BF_NP),
         chunk_major(vl.astype(np.float32)).astype(BF_NP)], axis=1)

    # ---- stage 1 on device
    nc1 = _get_prog("s1", build_stage1)
    r1 = run_bass_kernel_spmd(
        nc1, [{"xbig": xbs[c], "vhl": vhl} for c in core_ids], core_ids, trace=trace)
    xvs = [r1.results[c]["xv"] for c in core_ids]  # [R, SSH]
    css = [r1.results[c]["cs"] for c in core_ids]

    # ---- routing on host (tiny); cs = [ScalarE | VectorE spans], add both
    def core_colsum(csm):
        m = csm.astype(np.float64)
        return (m[:, :NI_CH] + m[:, NI_CH:]).T.reshape(IN)

    colsum = np.stack([
        core_colsum(css[2 * b]) + core_colsum(css[2 * b + 1]) for b in range(B)
    ])
    UL = _routing_host(colsum, x[:, -1, :], V_shared, U_shared, core_pool,
                       core_keys, gate_w, gate_b)

    # ---- stage 2 inputs: single bf16 rounds of xv and UL^T
    xvqs, ulqs = [], []
    for c in range(NCORES):
        xvqs.append(xvs[c].astype(BF_NP))                    # [R, SSH]
        ulqs.append(np.ascontiguousarray(
            UL[c // 2].T.astype(np.float32)).astype(BF_NP))  # [R, OUT]

    nc2 = _get_prog("s2", build_stage2)
    r2 = run_bass_kernel_spmd(
        nc2, [{"xvq": xvqs[c], "ulq": ulqs[c]} for c in core_ids], core_ids,
        trace=trace)
    # un-permute odev [128, 16*OUT] -> [SSH, OUT]: row sc*128+p = odev[p, sc]
    outs = [
        np.ascontiguousarray(
            r2.results[c]["odev"].reshape(128, SSH // 128, OUT)
            .transpose(1, 0, 2).reshape(SSH, OUT)).astype(np.float32)
        for c in core_ids]

    if trace:
        LAST_STATS.clear()
        LAST_STATS["stage1_ns"] = r1.exec_time_ns
        LAST_STATS["stage2_ns"] = r2.exec_time_ns
        LAST_STATS["total_ns"] = (
            (r1.exec_time_ns or 0) + (r2.exec_time_ns or 0)
            if (r1.exec_time_ns or r2.exec_time_ns) else None)

    return np.stack(
        [np.concatenate([outs[2 * b], outs[2 * b + 1]], axis=0) for b in range(B)]
    )
